# revision 1
# baseline (speedup 1.0000x reference)
"""BertBlock (mean-only LN, 16-head attention, relu FF) on 8 trn2 NeuronCores.

Sharding: head-parallel attention (2 heads / core) + sequence-parallel
norms & FF (512 rows / core). Collectives: one AllGather of the normed
input (transposed layout) + one ReduceScatter after out-proj. FF runs
fully locally on each core's row shard with full (host-pre-transposed)
FF weights streamed from HBM, so no collective is needed after FF2.

All heavy matmuls run as float32r (fast fp32 mode, ~1e-4 rel err).
"""
import sys

sys.path.insert(0, '/opt/trn_rl_repo')

import numpy as np
import concourse.bass as bass
from concourse import bacc
import concourse.mybir as mybir
import concourse.tile as tile
from concourse.masks import make_identity

S = 4096          # sequence length
H = 1024          # hidden
I_ = 4096         # ffn inner
NH = 16           # heads
HD = 64           # head dim
NC = 8            # cores
SM = S // NC      # 512 rows per core
DM = 128          # inner dims per core (2 heads x 64)
HC = H // 128     # 8 hidden chunks
ST = S // 512     # 8 s-tiles of 512
F32 = mybir.dt.float32
F32R = mybir.dt.float32r
BF16 = mybir.dt.bfloat16
AF = mybir.ActivationFunctionType
ALU = mybir.AluOpType
AXX = mybir.AxisListType.X

_CACHE = {}


def build_nc():
    nc = bacc.Bacc(None, target_bir_lowering=False, debug=False)
    P = lambda name, shape: nc.declare_dram_parameter(name, shape, F32, isOutput=False)
    x_m = P("x_m", [SM, H])
    wqkvT = P("wqkvT", [H, 3 * DM])      # [h, q|k|v cols of my 2 heads]
    bqkv = P("bqkv", [1, 3 * DM])
    owT = P("owT", [DM, H])              # o_w[:, my_cols].T
    ob = P("ob", [1, H])
    anw, anb = P("anw", [1, H]), P("anb", [1, H])
    fnw, fnb = P("fnw", [1, H]), P("fnb", [1, H])
    ff1wT = P("ff1wT", [H, I_])
    ff1b = P("ff1b", [32, 128])
    ff2wT = P("ff2wT", [I_, H])
    ffb2 = P("ffb2", [1, H])
    y = nc.declare_dram_parameter("y", [SM, H], F32, isOutput=True)

    with tile.TileContext(nc) as tc:
        cst = tc.alloc_tile_pool(name="cst", bufs=1)
        dram = tc.alloc_tile_pool(name="dram", bufs=1, space="DRAM")
        xmp = tc.alloc_tile_pool(name="xmp", bufs=1)
        setp = tc.alloc_tile_pool(name="setp", bufs=1)
        ps_set = tc.alloc_tile_pool(name="ps_set", bufs=2, space="PSUM")

        ag_in = dram.tile([H, SM], F32)
        ag_out = dram.tile([NC, H, SM], F32, addr_space="Shared")
        rs_in = dram.tile([S, H], F32)
        rs_out = dram.tile([SM, H], F32)

        # ---- constants ----
        ident = cst.tile([128, 128], F32)
        make_identity(nc, ident)
        ones_f = cst.tile([1, 128], F32)
        nc.gpsimd.memset(ones_f, 1.0)
        ones1 = cst.tile([1, 128], F32R)
        nc.vector.tensor_copy(ones1[:], ones_f[:])
        ones_col = cst.tile([128, 1], F32)
        nc.gpsimd.memset(ones_col, 1.0)

        def load_vec(p):
            t = setp.tile([1, H], F32, tag=f"v_{p.name}")
            nc.sync.dma_start(out=t[:], in_=p[:])
            return t

        vecs = {n: load_vec(p) for n, p in
                [("anw", anw), ("anb", anb), ("fnw", fnw), ("fnb", fnb),
                 ("ob", ob), ("ffb2", ffb2)]}

        def bcast(name, pool=None):
            # [1, H] -> [128, H] broadcast across partitions via PE
            v = vecs[name]
            bc = (pool or cst).tile([128, H], F32, tag=f"bc_{name}", name=f"bc_{name}")
            for hf in range(H // 512):
                ps = ps_set.tile([128, 512], F32)
                nc.tensor.matmul(ps[:], ones_f[0:1, :], v[0:1, hf * 512:(hf + 1) * 512],
                                 start=True, stop=True)
                nc.vector.tensor_copy(bc[:, hf * 512:(hf + 1) * 512], ps[:])
            return bc

        anw_bc, anb_bc = bcast("anw", setp), bcast("anb", setp)
        fnw_bc, fnb_bc = bcast("fnw"), bcast("fnb")
        ob_bc, ffb2_bc = bcast("ob"), bcast("ffb2")

        bqkv_sb = setp.tile([1, 3 * DM], F32)
        nc.sync.dma_start(out=bqkv_sb[:], in_=bqkv[:])
        qkvb_pp = []
        for j in range(3):
            ps = ps_set.tile([128, 512], F32)
            nc.tensor.matmul(ps[:, 0:1],
                             bqkv_sb[0:1, j * 128:(j + 1) * 128],
                             ones_f[0:1, 0:1], start=True, stop=True)
            t = cst.tile([128, 1], F32, tag=f"b_pp{j}")
            nc.vector.tensor_copy(t[:], ps[:, 0:1])
            qkvb_pp.append(t)

        ffb1_ld = setp.tile([32, 128], F32)
        nc.sync.dma_start(out=ffb1_ld[:], in_=ff1b[:])
        ps = ps_set.tile([128, 512], F32)
        nc.tensor.transpose(ps[:, 0:32], ffb1_ld[:], ident[0:32, 0:32])
        ffb1_pp = cst.tile([128, 32], F32)
        nc.vector.tensor_copy(ffb1_pp[:], ps[:, 0:32])

        # ---- LN1 on my rows + transpose + AllGather ----
        xm_tiles = []
        for i in range(4):
            t = xmp.tile([128, H], F32, tag=f"xm{i}")
            nc.sync.dma_start(out=t[:], in_=x_m[i * 128:(i + 1) * 128, :])
            xm_tiles.append(t)

        with tc.tile_pool(name="ln1", bufs=1) as lnp, \
             tc.tile_pool(name="ln1s", bufs=3) as lnsp:
            xn_tiles = []
            for i in range(4):
                ns = lnsp.tile([128, 1], F32, tag="negsum")
                nc.vector.reduce_sum(out=ns[:], in_=xm_tiles[i][:], axis=AXX,
                                     negate=True)
                nm = lnsp.tile([128, 1], F32, tag="negmean")
                nc.scalar.mul(nm[:], ns[:], 1.0 / H)
                xn = lnp.tile([128, H], F32, tag=f"xn{i}")
                nc.vector.scalar_tensor_tensor(
                    out=xn[:], in0=xm_tiles[i][:], scalar=nm[:], in1=anw_bc[:],
                    op0=ALU.add, op1=ALU.mult)
                nc.vector.tensor_add(xn[:], xn[:], anb_bc[:])
                xn_tiles.append(xn)
            for hc in range(HC):
                xt = lnp.tile([128, SM], F32, tag=f"xnT{hc}")
                for si in range(4):
                    ps = ps_set.tile([128, 512], F32, tag="tps")
                    nc.tensor.transpose(ps[:, 0:128],
                                        xn_tiles[si][:, hc * 128:(hc + 1) * 128],
                                        ident[:])
                    nc.vector.tensor_copy(xt[:, si * 128:(si + 1) * 128],
                                          ps[:, 0:128])
                nc.sync.dma_start(out=ag_in[hc * 128:(hc + 1) * 128, :], in_=xt[:])
            nc.gpsimd.collective_compute(
                "AllGather", ALU.bypass, replica_groups=[list(range(NC))],
                ins=[ag_in.opt()], outs=[ag_out.opt()])
        ps_set.release()
        setp.release()

        # ---- attention ----
        with tc.tile_pool(name="attn", bufs=1) as at, \
             tc.tile_pool(name="stream", bufs=10) as stp, \
             tc.tile_pool(name="vtp", bufs=3) as vtp, \
             tc.tile_pool(name="expp", bufs=6) as expp, \
             tc.tile_pool(name="rcp", bufs=2) as rcp, \
             tc.tile_pool(name="aop", bufs=3) as aop, \
             tc.tile_pool(name="ps_mm", bufs=3, space="PSUM") as ps_mm, \
             tc.tile_pool(name="ps_acc", bufs=1, space="PSUM") as ps_acc, \
             tc.tile_pool(name="ps_bc", bufs=1, space="PSUM") as ps_bc:

            wqkv_t = []
            for hc in range(HC):
                t = at.tile([128, 3 * DM], F32R, tag=f"wqkv{hc}")
                nc.sync.dma_start(out=t[:],
                                  in_=wqkvT[hc * 128:(hc + 1) * 128, :].bitcast(F32R))
                wqkv_t.append(t)
            owT_sb = at.tile([DM, H], F32R, tag="owT")
            nc.sync.dma_start(out=owT_sb[:], in_=owT[:].bitcast(F32R))

            QTp = [at.tile([128, S], BF16, tag=f"QTp{h}", name=f"QTp{h}")
                   for h in range(2)]
            KTp = [at.tile([128, S], BF16, tag=f"KTp{h}", name=f"KTp{h}")
                   for h in range(2)]
            for h in range(2):
                z = slice(HD, 128) if h == 0 else slice(0, HD)
                nc.gpsimd.memset(QTp[h][z, :], 0.0)
                nc.gpsimd.memset(KTp[h][z, :], 0.0)
            vaug = [[at.tile([128, HD + 1], BF16, tag=f"va{h}_{t}", name=f"va{h}_{t}")
                     for t in range(32)] for h in range(2)]
            for h in range(2):
                for t in range(32):
                    nc.vector.tensor_copy(vaug[h][t][:, HD:HD + 1], ones_col[:])

            for r in range(ST):
                xnr = []
                for hc in range(HC):
                    t = stp.tile([128, 512], F32R, tag="xnr")
                    nc.sync.dma_start(
                        out=t[:], in_=ag_out[r, hc * 128:(hc + 1) * 128, :].bitcast(F32R))
                    xnr.append(t)
                for j, dest in ((0, QTp), (1, KTp)):
                    ps = ps_mm.tile([128, 512], F32, tag="mm")
                    for hc in range(HC):
                        nc.tensor.matmul(ps[:], wqkv_t[hc][:, j * 128:(j + 1) * 128],
                                         xnr[hc][:], start=(hc == 0), stop=(hc == 7))
                    for h in range(2):
                        hs = slice(h * HD, (h + 1) * HD)
                        nc.vector.tensor_scalar_add(
                            dest[h][hs, r * 512:(r + 1) * 512], ps[hs, :],
                            qkvb_pp[j][hs, :])
                ps = ps_mm.tile([128, 512], F32, tag="mm")
                for hc in range(HC):
                    nc.tensor.matmul(ps[:], wqkv_t[hc][:, 2 * 128:3 * 128],
                                     xnr[hc][:], start=(hc == 0), stop=(hc == 7))
                vtmp = vtp.tile([128, 512], F32, tag="vtmp")
                nc.vector.tensor_scalar_add(vtmp[:], ps[:], qkvb_pp[2][:])
                for tb in range(4):
                    pst = ps_bc.tile([128, 128], F32, tag="vtp")
                    nc.tensor.transpose(pst[:], vtmp[:, tb * 128:(tb + 1) * 128],
                                        ident[:])
                    ti = r * 4 + tb
                    nc.vector.tensor_copy(vaug[0][ti][:, 0:HD], pst[:, 0:HD])
                    nc.vector.tensor_copy(vaug[1][ti][:, 0:HD], pst[:, HD:2 * HD])

            ctxT = at.tile([128, S], F32R, tag="ctxT")
            chains = [(h, r) for h in range(2) for r in range(ST)]
            for g in range(0, len(chains), 3):
                grp = chains[g:g + 3]
                cps_l = [ps_acc.tile([128, 512], F32, name=f"cps{g}_{i}",
                                     tag=f"cps{i}")
                         for i in range(len(grp))]
                for t in range(32):
                    exl = []
                    for (h, r), cps in zip(grp, cps_l):
                        sps = ps_mm.tile([128, 512], F32, tag="mm")
                        nc.tensor.matmul(sps[:],
                                         KTp[h][:, t * 128:(t + 1) * 128],
                                         QTp[h][:, r * 512:(r + 1) * 512],
                                         start=True, stop=True)
                        ex = expp.tile([128, 512], BF16, tag="exp")
                        nc.scalar.activation(ex[:], sps[:], AF.Exp, scale=0.125)
                        exl.append(ex)
                    for (h, r), cps, ex in zip(grp, cps_l, exl):
                        nc.tensor.matmul(cps[0:HD + 1, :], vaug[h][t][:, :],
                                         ex[:], start=(t == 0), stop=(t == 31),
                                         skip_group_check=True)
                for (h, r), cps in zip(grp, cps_l):
                    hs = slice(h * HD, (h + 1) * HD)
                    rc = rcp.tile([1, 512], F32R, tag="rc")
                    with nc.allow_low_precision(reason="f32r softmax denom"):
                        nc.vector.reciprocal(rc[:], cps[HD:HD + 1, :])
                    bps = ps_bc.tile([128, 512], F32, tag="rbc")
                    nc.tensor.matmul(bps[0:HD, :], ones1[0:1, 0:HD], rc[0:1, :],
                                     start=True, stop=True)
                    bsb = rcp.tile([HD, 512], F32, tag="bsb")
                    nc.vector.tensor_copy(bsb[:], bps[0:HD, :])
                    nc.vector.tensor_mul(ctxT[hs, r * 512:(r + 1) * 512],
                                         cps[0:HD, :], bsb[:])

            for s128 in range(32):
                ao = aop.tile([128, H], F32, tag="ao")
                for hf in range(2):
                    ps = ps_mm.tile([128, 512], F32, tag="mm")
                    nc.tensor.matmul(ps[:], ctxT[:, s128 * 128:(s128 + 1) * 128],
                                     owT_sb[:, hf * 512:(hf + 1) * 512],
                                     start=True, stop=True)
                    nc.vector.tensor_copy(ao[:, hf * 512:(hf + 1) * 512], ps[:])
                nc.sync.dma_start(out=rs_in[s128 * 128:(s128 + 1) * 128, :], in_=ao[:])

        nc.gpsimd.collective_compute(
            "ReduceScatter", ALU.add, replica_groups=[list(range(NC))],
            ins=[rs_in.opt()], outs=[rs_out.opt()])

        # ---- x2 = rs_out + x + o_b ; LN2 ; FF (local rows) ----
        with tc.tile_pool(name="ff", bufs=1) as ff, \
             tc.tile_pool(name="ffs", bufs=3) as ffsp, \
             tc.tile_pool(name="w1p", bufs=9) as w1p, \
             tc.tile_pool(name="w2p", bufs=4) as w2p, \
             tc.tile_pool(name="ps_f1", bufs=2, space="PSUM") as ps_f1, \
             tc.tile_pool(name="ps_f2", bufs=1, space="PSUM") as ps_f2:

            ln2p = tc.alloc_tile_pool(name="ln2p", bufs=1)
            x2_tiles, xn2_tiles = [], []
            for i in range(4):
                rl = ffsp.tile([128, H], F32, tag="rsld")
                nc.sync.dma_start(out=rl[:], in_=rs_out[i * 128:(i + 1) * 128, :])
                x2 = ff.tile([128, H], F32, tag=f"x2{i}")
                nc.vector.tensor_add(x2[:], rl[:], xm_tiles[i][:])
                nc.vector.tensor_add(x2[:], x2[:], ob_bc[:])
                x2_tiles.append(x2)
                ns = ffsp.tile([128, 1], F32, tag="negsum2")
                nc.vector.reduce_sum(out=ns[:], in_=x2[:], axis=AXX, negate=True)
                nm = ffsp.tile([128, 1], F32, tag="negmean2")
                nc.scalar.mul(nm[:], ns[:], 1.0 / H)
                xn2 = ln2p.tile([128, H], F32, tag=f"xn2{i}")
                nc.vector.scalar_tensor_tensor(
                    out=xn2[:], in0=x2[:], scalar=nm[:], in1=fnw_bc[:],
                    op0=ALU.add, op1=ALU.mult)
                nc.vector.tensor_add(xn2[:], xn2[:], fnb_bc[:])
                xn2_tiles.append(xn2)

            xn2T = []
            for hc in range(HC):
                xt = ff.tile([128, SM], F32R, tag=f"xn2T{hc}")
                for si in range(4):
                    ps = ps_f1.tile([128, 512], F32, tag="f1")
                    nc.tensor.transpose(ps[:, 0:128],
                                        xn2_tiles[si][:, hc * 128:(hc + 1) * 128],
                                        ident[:])
                    nc.vector.tensor_copy(xt[:, si * 128:(si + 1) * 128],
                                          ps[:, 0:128])
                xn2T.append(xt)
            ln2p.release()

            hT = [ff.tile([128, SM], F32R, tag=f"hT{i}", name=f"hT{i}") for i in range(32)]
            for ib in range(8):
                w1t = []
                for hc in range(HC):
                    t = w1p.tile([128, 512], F32R, tag="w1")
                    nc.sync.dma_start(
                        out=t[:],
                        in_=ff1wT[hc * 128:(hc + 1) * 128,
                                  ib * 512:(ib + 1) * 512].bitcast(F32R))
                    w1t.append(t)
                for sub in range(4):
                    it = ib * 4 + sub
                    ps = ps_f1.tile([128, 512], F32, tag="f1")
                    for hc in range(HC):
                        nc.tensor.matmul(ps[:],
                                         w1t[hc][:, sub * 128:(sub + 1) * 128],
                                         xn2T[hc][:], start=(hc == 0), stop=(hc == 7))
                    nc.scalar.activation(hT[it][:], ps[:], AF.Relu,
                                         bias=ffb1_pp[:, it:it + 1])

            y_sb = [ff.tile([128, H], F32, tag=f"y{i}", name=f"ysb{i}") for i in range(4)]
            for hf in range(2):
                yps = [ps_f2.tile([128, 512], F32, name=f"yps{hf}_{i}", tag=f"yps{i}", bufs=1) for i in range(4)]
                for ic in range(32):
                    w2t = w2p.tile([128, 512], F32R, tag="w2")
                    nc.sync.dma_start(
                        out=w2t[:],
                        in_=ff2wT[ic * 128:(ic + 1) * 128,
                                  hf * 512:(hf + 1) * 512].bitcast(F32R))
                    for s4 in range(4):
                        nc.tensor.matmul(yps[s4][:],
                                         hT[ic][:, s4 * 128:(s4 + 1) * 128],
                                         w2t[:], start=(ic == 0), stop=(ic == 31),
                                         skip_group_check=True)
                for s4 in range(4):
                    sl = slice(hf * 512, (hf + 1) * 512)
                    nc.vector.tensor_add(y_sb[s4][:, sl], yps[s4][:],
                                         x2_tiles[s4][:, sl])
                    nc.vector.tensor_add(y_sb[s4][:, sl], y_sb[s4][:, sl],
                                         ffb2_bc[:, sl])
            for s4 in range(4):
                nc.sync.dma_start(out=y[s4 * 128:(s4 + 1) * 128, :], in_=y_sb[s4][:])

        xmp.release()
        dram.release()
        cst.release()

    nc.compile()
    return nc


def make_in_maps(inputs):
    f = lambda a: np.ascontiguousarray(np.asarray(a, dtype=np.float32))
    x = f(inputs["x"])
    q_w, k_w, v_w = f(inputs["q_w"]), f(inputs["k_w"]), f(inputs["v_w"])
    o_w = f(inputs["o_w"])
    ff1_w, ff2_w = f(inputs["ff1_w"]), f(inputs["ff2_w"])
    ff1wT = np.ascontiguousarray(ff1_w.T)
    ff2wT = np.ascontiguousarray(ff2_w.T)
    ff1b = np.ascontiguousarray(f(inputs["ff1_b"]).reshape(32, 128))
    row = lambda a: np.ascontiguousarray(a.reshape(1, -1))
    in_maps = []
    for m in range(NC):
        dm = slice(m * DM, (m + 1) * DM)
        wqkvT = np.ascontiguousarray(
            np.concatenate([q_w[dm].T, k_w[dm].T, v_w[dm].T], axis=1))
        bqkv = np.ascontiguousarray(np.concatenate(
            [f(inputs["q_b"])[dm], f(inputs["k_b"])[dm], f(inputs["v_b"])[dm]]
        ).reshape(1, -1))
        in_maps.append({
            "x_m": np.ascontiguousarray(x[m * SM:(m + 1) * SM]),
            "wqkvT": wqkvT,
            "bqkv": bqkv,
            "owT": np.ascontiguousarray(o_w[:, dm].T),
            "ob": row(f(inputs["o_b"])),
            "anw": row(f(inputs["an_w"])), "anb": row(f(inputs["an_b"])),
            "fnw": row(f(inputs["fn_w"])), "fnb": row(f(inputs["fn_b"])),
            "ff1wT": ff1wT, "ff1b": ff1b,
            "ff2wT": ff2wT, "ffb2": row(f(inputs["ff2_b"])),
        })
    return in_maps


def kernel(**inputs) -> np.ndarray:
    from concourse.bass_utils import run_bass_kernel_spmd
    if "nc" not in _CACHE:
        _CACHE["nc"] = build_nc()
    nc = _CACHE["nc"]
    in_maps = make_in_maps(inputs)
    res = run_bass_kernel_spmd(nc, in_maps, core_ids=list(range(NC)))
    return np.concatenate([res.results[m]["y"] for m in range(NC)], axis=0)



# revision 18
# speedup vs baseline: 1.4269x; 1.4269x over previous
"""BertBlock (mean-only LN, 16-head attention, relu FF) on 8 trn2 NeuronCores.

Sharding: head-parallel attention (2 heads / core) + sequence-parallel FF
(512 rows / core, scattered in 4x128-row blocks to match ReduceScatter chunk
placement). No AllGather: every core reads the full transposed input xT
(bf16) and computes LN1 means locally on the PE (mean folded into the QKV
matmul as a rank-1 correction; LN scale/bias folded into weights on host).
One bf16 ReduceScatter after out-proj, split into 4 row-chunks overlapped
under attention compute. FF runs locally with streamed bf16 weights.
"""
import sys

sys.path.insert(0, '/opt/trn_rl_repo')

import numpy as np
import ml_dtypes
import concourse.bass as bass
from concourse import bacc
import concourse.mybir as mybir
import concourse.tile as tile
from concourse.masks import make_identity

S = 4096          # sequence length
H = 1024          # hidden
I_ = 4096         # ffn inner
NH = 16           # heads
HD = 64           # head dim
NC = 8            # cores
SM = S // NC      # 512 rows per core
DM = 128          # inner dims per core (2 heads x 64)
HC = H // 128     # 8 hidden chunks
ST = S // 512     # 8 s-tiles of 512
RQ = 4            # ReduceScatter chunks
F32 = mybir.dt.float32
F32R = mybir.dt.float32r
BF16 = mybir.dt.bfloat16
AF = mybir.ActivationFunctionType
ALU = mybir.AluOpType
AXX = mybir.AxisListType.X

_CACHE = {}
DEBUG = False


def build_nc():
    nc = bacc.Bacc(None, target_bir_lowering=False, debug=False)
    P = lambda name, shape, dt=F32: nc.declare_dram_parameter(name, shape, dt,
                                                              isOutput=False)
    xT = P("xT", [H, S], BF16)           # full normed-input-free transposed x
    wqkvT = P("wqkvT", [H, 3 * DM], BF16)  # anw-folded [h, q|k|v of my 2 heads]
    nrw = P("nrw", [1, 3 * DM])          # -colsums of wqkvT (mean correction)
    bqkv = P("bqkv", [1, 3 * DM])        # anb-folded biases
    owT = P("owT", [DM, H], BF16)        # o_w[:, my_cols].T
    ob = P("ob", [1, H])
    ff1wT = P("ff1wT", [H, I_], BF16)    # fnw-folded
    ff1b = P("ff1b", [32, 128])          # fnb-folded
    ff2wT = P("ff2wT", [I_, H], BF16)
    ffb2 = P("ffb2", [1, H])
    x_res = P("x_res", [SM, H])          # my owned rows (4 scattered 128-blocks)
    y = nc.declare_dram_parameter("y", [SM, H], F32, isOutput=True)
    if DEBUG:
        dbg_qt = nc.declare_dram_parameter("dbg_qt", [128, S], BF16, isOutput=True)
        dbg_kt = nc.declare_dram_parameter("dbg_kt", [128, S], BF16, isOutput=True)
        dbg_ctx = nc.declare_dram_parameter("dbg_ctx", [128, S], BF16, isOutput=True)
        dbg_rs = nc.declare_dram_parameter("dbg_rs", [SM, H], BF16, isOutput=True)

    with tile.TileContext(nc) as tc:
        cst = tc.alloc_tile_pool(name="cst", bufs=1)
        dram = tc.alloc_tile_pool(name="dram", bufs=1, space="DRAM")
        setp = tc.alloc_tile_pool(name="setp", bufs=1)
        ps_set = tc.alloc_tile_pool(name="ps_set", bufs=2, space="PSUM")

        rs_in = dram.tile([S, H], BF16)
        rs_out = dram.tile([SM, H], BF16)

        # ---- constants ----
        ident_f = cst.tile([128, 128], F32)
        make_identity(nc, ident_f)
        ones_f = cst.tile([1, 128], F32)
        nc.gpsimd.memset(ones_f, 1.0)
        ones1 = cst.tile([1, 128], F32R)
        nc.vector.tensor_copy(ones1[:], ones_f[:])
        ones_col_b = cst.tile([128, 1], BF16)
        nc.gpsimd.memset(ones_col_b, 1.0)

        def load_vec(p):
            t = setp.tile([1, H], F32, tag=f"v_{p.name}")
            nc.sync.dma_start(out=t[:], in_=p[:])
            return t

        vecs = {n: load_vec(p) for n, p in [("ob", ob), ("ffb2", ffb2)]}

        def bcast(name):
            # [1, H] -> [128, H] broadcast across partitions via PE
            v = vecs[name]
            bc = cst.tile([128, H], F32, tag=f"bc_{name}", name=f"bc_{name}")
            for hf in range(H // 512):
                ps = ps_set.tile([128, 512], F32)
                nc.tensor.matmul(ps[:], ones_f[0:1, :], v[0:1, hf * 512:(hf + 1) * 512],
                                 start=True, stop=True)
                nc.vector.tensor_copy(bc[:, hf * 512:(hf + 1) * 512], ps[:])
            return bc

        ob_bc, ffb2_bc = bcast("ob"), bcast("ffb2")

        # qkv biases as per-partition columns [128, 1] x3
        bqkv_sb = setp.tile([1, 3 * DM], F32)
        nc.sync.dma_start(out=bqkv_sb[:], in_=bqkv[:])
        qkvb_pp = []
        for j in range(3):
            ps = ps_set.tile([128, 512], F32)
            nc.tensor.matmul(ps[:, 0:1],
                             bqkv_sb[0:1, j * 128:(j + 1) * 128],
                             ones_f[0:1, 0:1], start=True, stop=True)
            t = cst.tile([128, 1], F32, tag=f"b_pp{j}")
            nc.vector.tensor_copy(t[:], ps[:, 0:1])
            qkvb_pp.append(t)

        # neg row-sums of wqkv as f32r rows for the mean-correction matmul
        nrw_ld = setp.tile([1, 3 * DM], F32)
        nc.sync.dma_start(out=nrw_ld[:], in_=nrw[:])
        nrw_r = cst.tile([1, 3 * DM], F32R)
        nc.vector.tensor_copy(nrw_r[:], nrw_ld[:])

        # ff1 bias transposed to per-partition layout [128, 32]
        ffb1_ld = setp.tile([32, 128], F32)
        nc.sync.dma_start(out=ffb1_ld[:], in_=ff1b[:])
        ps = ps_set.tile([128, 512], F32)
        nc.tensor.transpose(ps[:, 0:32], ffb1_ld[:], ident_f[0:32, 0:32])
        ffb1_pp = cst.tile([128, 32], F32)
        nc.vector.tensor_copy(ffb1_pp[:], ps[:, 0:32])
        ps_set.release()
        setp.release()

        # ---- attention state tiles ----
        at = tc.alloc_tile_pool(name="at", bufs=1)
        wqkv_t = []
        for hc in range(HC):
            t = at.tile([128, 3 * DM], BF16, tag=f"wqkv{hc}")
            nc.sync.dma_start(out=t[:], in_=wqkvT[hc * 128:(hc + 1) * 128, :])
            wqkv_t.append(t)
        owT_sb = at.tile([DM, H], BF16, tag="owT")
        nc.sync.dma_start(out=owT_sb[:], in_=owT[:])

        QTp = [at.tile([128, S], BF16, tag=f"QTp{h}", name=f"QTp{h}")
               for h in range(2)]
        KTp = [at.tile([128, S], BF16, tag=f"KTp{h}", name=f"KTp{h}")
               for h in range(2)]
        for h in range(2):
            z = slice(HD, 128) if h == 0 else slice(0, HD)
            nc.gpsimd.memset(QTp[h][z, :], 0.0)
            nc.gpsimd.memset(KTp[h][z, :], 0.0)
        vaug = [[at.tile([128, HD + 1], BF16, tag=f"va{h}_{t}", name=f"va{h}_{t}")
                 for t in range(32)] for h in range(2)]
        for h in range(2):
            for t in range(32):
                nc.vector.tensor_copy(vaug[h][t][:, HD:HD + 1], ones_col_b[:])
        ctxT = at.tile([128, S], BF16, tag="ctxT")

        # ---- phase A: stream xT, compute means on PE, QKV with rank-1
        # mean correction folded into the matmul ----
        with tc.tile_pool(name="xtp", bufs=2) as xtp, \
             tc.tile_pool(name="musb", bufs=3) as musb, \
             tc.tile_pool(name="vtp", bufs=3) as vtp, \
             tc.tile_pool(name="ps_mu", bufs=2, space="PSUM") as ps_mu, \
             tc.tile_pool(name="ps_qkv", bufs=3, space="PSUM") as ps_qkv, \
             tc.tile_pool(name="ps_vt", bufs=2, space="PSUM") as ps_vt:
            for r in range(ST):
                xtr = []
                for hc in range(HC):
                    t = xtp.tile([128, 512], BF16, tag=f"xtr{hc}")
                    nc.sync.dma_start(
                        out=t[:], in_=xT[hc * 128:(hc + 1) * 128,
                                         r * 512:(r + 1) * 512])
                    xtr.append(t)
                pmu = ps_mu.tile([1, 512], F32, tag="mu")
                for hc in range(HC):
                    nc.tensor.matmul(pmu[:], ones_col_b[:], xtr[hc][:],
                                     start=(hc == 0), stop=(hc == 7))
                mu = musb.tile([1, 512], F32R, tag="mu_sb")
                nc.vector.tensor_copy(mu[:], pmu[:])
                for j in range(3):
                    psq = ps_qkv.tile([128, 512], F32, tag="qkv")
                    for hc in range(HC):
                        nc.tensor.matmul(psq[:],
                                         wqkv_t[hc][:, j * 128:(j + 1) * 128],
                                         xtr[hc][:], start=(hc == 0), stop=False)
                    # += (-rowsum_j/H) outer mu  (removes the mean term)
                    nc.tensor.matmul(psq[:], nrw_r[0:1, j * 128:(j + 1) * 128],
                                     mu[:], start=False, stop=True)
                    if j < 2:
                        dest = QTp if j == 0 else KTp
                        for h in range(2):
                            hs = slice(h * HD, (h + 1) * HD)
                            nc.scalar.activation(
                                dest[h][hs, r * 512:(r + 1) * 512], psq[hs, :],
                                AF.Identity, bias=qkvb_pp[j][hs, :])
                    else:
                        vtmp = vtp.tile([128, 512], F32, tag="vtmp")
                        nc.scalar.activation(vtmp[:], psq[:], AF.Identity,
                                             bias=qkvb_pp[2][:])
                        for tb in range(4):
                            pst = ps_vt.tile([128, 128], F32, tag="vt")
                            nc.tensor.transpose(
                                pst[:], vtmp[:, tb * 128:(tb + 1) * 128],
                                ident_f[:])
                            ti = r * 4 + tb
                            nc.vector.tensor_copy(vaug[0][ti][:, 0:HD],
                                                  pst[:, 0:HD])
                            nc.vector.tensor_copy(vaug[1][ti][:, 0:HD],
                                                  pst[:, HD:2 * HD])

        # ---- phase B: attention (h-paired chains, out-proj + chunked RS
        # interleaved) ----
        with tc.tile_pool(name="expp", bufs=4) as expp, \
             tc.tile_pool(name="rcp", bufs=4) as rcp, \
             tc.tile_pool(name="aop", bufs=3) as aop, \
             tc.tile_pool(name="ps_sc", bufs=4, space="PSUM") as ps_sc, \
             tc.tile_pool(name="ps_cx", bufs=1, space="PSUM") as ps_cx, \
             tc.tile_pool(name="ps_o", bufs=1, space="PSUM") as ps_o:
            for r in range(ST):
                cps_l = [ps_cx.tile([128, 512], F32, name=f"cps{r}_{h}",
                                    tag=f"cps{h}") for h in range(2)]
                for t in range(32):
                    exl = []
                    for h in range(2):
                        sps = ps_sc.tile([128, 512], F32, tag="sc")
                        nc.tensor.matmul(sps[:],
                                         KTp[h][:, t * 128:(t + 1) * 128],
                                         QTp[h][:, r * 512:(r + 1) * 512],
                                         start=True, stop=True)
                        ex = expp.tile([128, 512], BF16, tag="exp")
                        nc.scalar.activation(ex[:], sps[:], AF.Exp, scale=0.125)
                        exl.append(ex)
                    for h, ex in enumerate(exl):
                        nc.tensor.matmul(cps_l[h][0:HD + 1, :],
                                         vaug[h][t][:, :], ex[:],
                                         start=(t == 0), stop=(t == 31),
                                         skip_group_check=True)
                for h in range(2):
                    hs = slice(h * HD, (h + 1) * HD)
                    den = rcp.tile([1, 512], F32, tag="den")
                    nc.vector.tensor_copy(den[:], cps_l[h][HD:HD + 1, :])
                    rc = rcp.tile([1, 512], F32, tag="rc")
                    nc.vector.reciprocal_approx_fast(out=rc[:], in_=den[:])
                    rc_r = rcp.tile([1, 512], F32R, tag="rc_r")
                    nc.vector.tensor_copy(rc_r[:], rc[:])
                    bps = ps_o.tile([128, 512], F32, tag="rbc")
                    nc.tensor.matmul(bps[0:HD, :], ones1[0:1, 0:HD],
                                     rc_r[:], start=True, stop=True)
                    bsb = rcp.tile([HD, 512], F32, tag="bsb")
                    nc.vector.tensor_copy(bsb[:], bps[0:HD, :])
                    nc.vector.tensor_mul(ctxT[hs, r * 512:(r + 1) * 512],
                                         cps_l[h][0:HD, :], bsb[:])
                for s128 in range(4 * r, 4 * r + 4):
                    ao = aop.tile([128, H], BF16, tag="ao")
                    for hf in range(2):
                        ps = ps_o.tile([128, 512], F32, tag="op")
                        nc.tensor.matmul(ps[:],
                                         ctxT[:, s128 * 128:(s128 + 1) * 128],
                                         owT_sb[:, hf * 512:(hf + 1) * 512],
                                         start=True, stop=True)
                        nc.vector.tensor_copy(ao[:, hf * 512:(hf + 1) * 512],
                                              ps[:])
                    nc.sync.dma_start(out=rs_in[s128 * 128:(s128 + 1) * 128, :],
                                      in_=ao[:])
                if r % 2 == 1:
                    q = r // 2
                    nc.gpsimd.collective_compute(
                        "ReduceScatter", ALU.add,
                        replica_groups=[list(range(NC))],
                        ins=[rs_in[q * 1024:(q + 1) * 1024, :].opt()],
                        outs=[rs_out[q * 128:(q + 1) * 128, :].opt()])
        if DEBUG:
            nc.sync.dma_start(out=dbg_qt[:], in_=QTp[0][:])
            nc.sync.dma_start(out=dbg_kt[:], in_=KTp[0][:])
            nc.sync.dma_start(out=dbg_ctx[:], in_=ctxT[:])
            nc.sync.dma_start(out=dbg_rs[:], in_=rs_out[:])
        at.release()

        # ---- phase C: x2 = rs_out + x_res + o_b ; LN2 (mean only, affine
        # folded into ff1) ; FF on my 512 scattered rows ----
        with tc.tile_pool(name="ff", bufs=1) as ff, \
             tc.tile_pool(name="ffs", bufs=4) as ffsp, \
             tc.tile_pool(name="w1p", bufs=9) as w1p, \
             tc.tile_pool(name="w2p", bufs=6) as w2p, \
             tc.tile_pool(name="ps_f1", bufs=2, space="PSUM") as ps_f1, \
             tc.tile_pool(name="ps_f2", bufs=1, space="PSUM") as ps_f2:

            ln2p = tc.alloc_tile_pool(name="ln2p", bufs=1)
            x2_tiles, xn2_tiles = [], []
            for i in range(4):
                rl = ffsp.tile([128, H], BF16, tag="rsld")
                nc.sync.dma_start(out=rl[:], in_=rs_out[i * 128:(i + 1) * 128, :])
                xr = ffsp.tile([128, H], F32, tag="xres")
                nc.sync.dma_start(out=xr[:], in_=x_res[i * 128:(i + 1) * 128, :])
                x2 = ff.tile([128, H], F32, tag=f"x2{i}")
                nc.vector.tensor_add(x2[:], rl[:], xr[:])
                nc.vector.tensor_add(x2[:], x2[:], ob_bc[:])
                x2_tiles.append(x2)
                ns = ffsp.tile([128, 1], F32, tag="negsum2")
                nc.vector.reduce_sum(out=ns[:], in_=x2[:], axis=AXX, negate=True)
                nm = ffsp.tile([128, 1], F32, tag="negmean2")
                nc.scalar.mul(nm[:], ns[:], 1.0 / H)
                xn2 = ln2p.tile([128, H], F32, tag=f"xn2{i}")
                nc.scalar.activation(xn2[:], x2[:], AF.Identity, bias=nm[:])
                xn2_tiles.append(xn2)

            xn2T = []
            for hc in range(HC):
                xt = ff.tile([128, SM], BF16, tag=f"xn2T{hc}")
                for si in range(4):
                    ps = ps_f1.tile([128, 128], F32, tag="tp")
                    nc.tensor.transpose(ps[:],
                                        xn2_tiles[si][:, hc * 128:(hc + 1) * 128],
                                        ident_f[:])
                    nc.vector.tensor_copy(xt[:, si * 128:(si + 1) * 128],
                                          ps[:])
                xn2T.append(xt)
            ln2p.release()

            hT = [ff.tile([128, SM], BF16, tag=f"hT{i}", name=f"hT{i}")
                  for i in range(32)]
            for ib in range(8):
                w1t = []
                for hc in range(HC):
                    t = w1p.tile([128, 512], BF16, tag="w1")
                    nc.sync.dma_start(
                        out=t[:],
                        in_=ff1wT[hc * 128:(hc + 1) * 128,
                                  ib * 512:(ib + 1) * 512])
                    w1t.append(t)
                for sub in range(4):
                    it = ib * 4 + sub
                    ps = ps_f1.tile([128, 512], F32, tag="f1")
                    for hc in range(HC):
                        nc.tensor.matmul(ps[:],
                                         w1t[hc][:, sub * 128:(sub + 1) * 128],
                                         xn2T[hc][:], start=(hc == 0),
                                         stop=(hc == 7))
                    nc.scalar.activation(hT[it][:], ps[:], AF.Relu,
                                         bias=ffb1_pp[:, it:it + 1])

            y_sb = [ff.tile([128, H], F32, tag=f"y{i}", name=f"ysb{i}")
                    for i in range(4)]
            for hf in range(2):
                yps = [ps_f2.tile([128, 512], F32, name=f"yps{hf}_{i}",
                                  tag=f"yps{i}", bufs=1) for i in range(4)]
                for ic in range(32):
                    w2t = w2p.tile([128, 512], BF16, tag="w2")
                    nc.sync.dma_start(
                        out=w2t[:],
                        in_=ff2wT[ic * 128:(ic + 1) * 128,
                                  hf * 512:(hf + 1) * 512])
                    for s4 in range(4):
                        nc.tensor.matmul(yps[s4][:],
                                         hT[ic][:, s4 * 128:(s4 + 1) * 128],
                                         w2t[:], start=(ic == 0), stop=(ic == 31),
                                         skip_group_check=True)
                for s4 in range(4):
                    sl = slice(hf * 512, (hf + 1) * 512)
                    nc.vector.tensor_add(y_sb[s4][:, sl], yps[s4][:],
                                         x2_tiles[s4][:, sl])
                    nc.vector.tensor_add(y_sb[s4][:, sl], y_sb[s4][:, sl],
                                         ffb2_bc[:, sl])
            for s4 in range(4):
                nc.sync.dma_start(out=y[s4 * 128:(s4 + 1) * 128, :],
                                  in_=y_sb[s4][:])

        dram.release()
        cst.release()

    nc.compile()
    return nc


def make_in_maps(inputs):
    bf16 = ml_dtypes.bfloat16
    f = lambda a: np.asarray(a, dtype=np.float32)
    x = f(inputs["x"])
    anw, anb = f(inputs["an_w"]), f(inputs["an_b"])
    fnw, fnb = f(inputs["fn_w"]), f(inputs["fn_b"])
    # fold LN1 affine into qkv weights/biases (exact algebra)
    q_w, k_w, v_w = f(inputs["q_w"]), f(inputs["k_w"]), f(inputs["v_w"])
    qkv_w = [w * anw[None, :] for w in (q_w, k_w, v_w)]
    qkv_b = [f(inputs[n]) + w0 @ anb
             for n, w0 in (("q_b", q_w), ("k_b", k_w), ("v_b", v_w))]
    o_w = f(inputs["o_w"])
    # fold LN2 affine into ff1
    ff1_w = f(inputs["ff1_w"]) * fnw[None, :]
    ff1_b = f(inputs["ff1_b"]) + f(inputs["ff1_w"]) @ fnb
    ff2_w = f(inputs["ff2_w"])

    xT = np.ascontiguousarray(x.T.astype(bf16))
    ff1wT = np.ascontiguousarray(ff1_w.T.astype(bf16))
    ff2wT = np.ascontiguousarray(ff2_w.T.astype(bf16))
    ff1b_t = np.ascontiguousarray(ff1_b.reshape(32, 128))
    row = lambda a: np.ascontiguousarray(a.reshape(1, -1))
    in_maps = []
    for m in range(NC):
        dm = slice(m * DM, (m + 1) * DM)
        wqkvT = np.concatenate([w[dm].T for w in qkv_w], axis=1)
        nrw = -wqkvT.sum(axis=0, keepdims=True) / H
        bqkv = np.concatenate([b[dm] for b in qkv_b]).reshape(1, -1)
        # rows owned by core m: for each RS chunk q, rows q*1024+m*128 ..+128
        own = np.concatenate([x[q * 1024 + m * 128: q * 1024 + (m + 1) * 128]
                              for q in range(RQ)], axis=0)
        in_maps.append({
            "xT": xT,
            "wqkvT": np.ascontiguousarray(wqkvT.astype(bf16)),
            "nrw": np.ascontiguousarray(nrw.astype(np.float32)),
            "bqkv": np.ascontiguousarray(bqkv.astype(np.float32)),
            "owT": np.ascontiguousarray(o_w[:, dm].T.astype(bf16)),
            "ob": row(f(inputs["o_b"])),
            "ff1wT": ff1wT, "ff1b": ff1b_t,
            "ff2wT": ff2wT, "ffb2": row(f(inputs["ff2_b"])),
            "x_res": np.ascontiguousarray(own),
        })
    return in_maps


def assemble(results):
    y = np.empty((S, H), dtype=np.float32)
    for m in range(NC):
        ym = results[m]["y"]
        for q in range(RQ):
            y[q * 1024 + m * 128: q * 1024 + (m + 1) * 128] = \
                ym[q * 128:(q + 1) * 128]
    return y


def kernel(**inputs) -> np.ndarray:
    from concourse.bass_utils import run_bass_kernel_spmd
    if "nc" not in _CACHE:
        _CACHE["nc"] = build_nc()
    nc = _CACHE["nc"]
    in_maps = make_in_maps(inputs)
    res = run_bass_kernel_spmd(nc, in_maps, core_ids=list(range(NC)))
    return assemble(res.results)
